# revision 2
# baseline (speedup 1.0000x reference)
"""Trainium2 Bass kernel for CAM (channel attention module).

Reference computation (per batch b):
    q = x_low[b]  as [C, N]   (C=512, N=64*64=4096)
    k = x_high[b] as [C, N]
    E = q @ k.T                              # [C, C]
    att = softmax(rowmax(E) - E, axis=-1)    # == exp(rowmin(E) - E) / Z
    out = gamma * (att @ k) + x_low[b]

Sharding: data-parallel over batch. 16 batches / 8 cores = 2 per core.
gamma is replicated (pre-broadcast on host to [128,1]).

Inputs are cast to fp16 on the host (numerically validated: max rel err
~8.7e-3 vs the fp32 reference, under the 2e-2 gate), halving HBM load
traffic. The n-major operand layouts needed by the first matmul
(contraction over N) are produced by DMA x-bar transposes (SBUF->SBUF,
fp16), freeing the PE to do only the two real matmuls plus the tiny
attention transpose. Matmuls run fp16 in / fp32 PSUM accumulation.
"""

import sys

sys.path.insert(0, "/opt/trn_rl_repo")

import numpy as np

B, C, H, W = 16, 512, 64, 64
N = H * W               # 4096
N_CORES = 8
B_LOC = B // N_CORES    # 2 batches per core
P = 128                 # partitions
CP = C // P             # 4 channel chunks
ST = 1024               # load sub-tile free size (fp16 words)
NS = N // ST            # 4 sub-tiles per (tensor, cc)
SC = ST // P            # 8 n-chunks per sub-tile
NN = N // P             # 32 n chunks of 128
FB = 512                # free-dim block (psum bank) for mm2 output
NB = N // FB            # 8 n blocks of 512

_CACHE = {}


def _build(reps=0, variant="full"):
    import contextlib
    import concourse.bacc as bacc
    import concourse.tile as tile
    import concourse.mybir as mybir
    from concourse.masks import make_identity

    f32 = mybir.dt.float32
    f16 = mybir.dt.float16

    nc = bacc.Bacc("TRN2", target_bir_lowering=False, debug=False)

    xh = nc.dram_tensor("xh", [B_LOC, C, N], f16, kind="ExternalInput")
    xl = nc.dram_tensor("xl", [B_LOC, C, N], f16, kind="ExternalInput")
    gm = nc.dram_tensor("gm", [P, 1], f32, kind="ExternalInput")
    out = nc.dram_tensor("out", [B_LOC, C, N], f32, kind="ExternalOutput")

    with tile.TileContext(nc) as tc:
        with (
            tc.tile_pool(name="const", bufs=1) as const_pool,
            tc.tile_pool(name="kn", bufs=NS * CP + 2) as kn_pool,
            tc.tile_pool(name="qn", bufs=NS * CP + 2) as qn_pool,
            tc.tile_pool(name="kT", bufs=NS + 1) as kT_pool,
            tc.tile_pool(name="qT", bufs=NS + 1) as qT_pool,
            tc.tile_pool(name="att", bufs=CP) as att_pool,
            tc.tile_pool(name="attT", bufs=CP) as attT_pool,
            tc.tile_pool(name="osb", bufs=8) as out_pool,
            tc.tile_pool(name="small", bufs=24) as small_pool,
            tc.tile_pool(name="psE", bufs=CP, space="PSUM") as psE_pool,
            tc.tile_pool(name="psT", bufs=2, space="PSUM") as psT_pool,
            tc.tile_pool(name="psA", bufs=2, space="PSUM") as psA_pool,
        ):
            ident_f = const_pool.tile([P, P], f32)
            make_identity(nc, ident_f[:])
            identh = const_pool.tile([P, P], f16)
            nc.vector.tensor_copy(identh[:], ident_f[:])
            gsb = const_pool.tile([P, 1], f32)
            nc.sync.dma_start(gsb[:], gm.ap())

            rep_ctx = tc.For_i(0, reps, 1) if reps else contextlib.nullcontext()
            with rep_ctx:
              for b in range(B_LOC):
                # ---- load natural fp16 sub-tiles [128, ST] ----
                KN = [[None] * NS for _ in range(CP)]
                QN = [[None] * NS for _ in range(CP)]
                KT = [None] * NS
                QT = [None] * NS
                for s in range(NS):
                    ssl = slice(s * ST, (s + 1) * ST)
                    for cc in range(CP):
                        csl = slice(cc * P, (cc + 1) * P)
                        kt = kn_pool.tile([P, ST], f16, tag="kn", name=f"kn{b}_{cc}_{s}")
                        qt = qn_pool.tile([P, ST], f16, tag="qn", name=f"qn{b}_{cc}_{s}")
                        nc.sync.dma_start(kt[:], xh.ap()[b, csl, ssl])
                        nc.sync.dma_start(qt[:], xl.ap()[b, csl, ssl])
                        KN[cc][s] = kt
                        QN[cc][s] = qt

                def qn_blk(cc, lo, width):
                    s = lo // ST
                    o = lo - s * ST
                    return QN[cc][s][:, o:o + width]

                def kn_blk(cc, lo, width):
                    s = lo // ST
                    o = lo - s * ST
                    return KN[cc][s][:, o:o + width]

                if variant == "dma":
                    # loads + stores only: measures the pure HBM floor
                    for nb in range(NB):
                        for ic in range(CP):
                            isl = slice(ic * P, (ic + 1) * P)
                            src = qn_blk(ic, nb * FB, FB) if nb % 2 == 0 else kn_blk(ic, nb * FB, FB)
                            osb = out_pool.tile([P, FB], f32, tag="osb")
                            nc.vector.tensor_copy(osb[:], src)
                            nc.gpsimd.dma_start(out.ap()[b, isl, nb * FB:(nb + 1) * FB], osb[:])
                    continue

                # ---- x-bar transposes: [128c, ST n] -> [128n, SC, 128c] ----
                # One s-group tile holds SC n-chunks for all CP channel chunks:
                # T[s][p, j, cc, c] = nat[cc*128+c, (s*SC+j)*128 + p]
                for s in range(NS):
                    kTs = kT_pool.tile([P, SC, CP, P], f16, tag="kT", name=f"kT{b}_{s}")
                    qTs = qT_pool.tile([P, SC, CP, P], f16, tag="qT", name=f"qT{b}_{s}")
                    for cc in range(CP):
                        nc.scalar.dma_start(kTs[:, :, cc, :], KN[cc][s][:], transpose=True)
                        nc.scalar.dma_start(qTs[:, :, cc, :], QN[cc][s][:], transpose=True)
                    KT[s] = kTs
                    QT[s] = qTs

                if variant == "dmax":
                    # loads + xbar transposes + stores: measures DMA concurrency
                    for nb in range(NB):
                        for ic in range(CP):
                            isl = slice(ic * P, (ic + 1) * P)
                            s = nb // 2
                            src = QT[s][:, 0, ic, :] if nb % 2 == 0 else KT[s][:, 0, ic, :]
                            osb = out_pool.tile([P, FB], f32, tag="osb")
                            nc.vector.tensor_copy(osb[:, 0:P], src)
                            nc.vector.tensor_copy(osb[:, P:2 * P], src)
                            nc.vector.tensor_copy(osb[:, 2 * P:3 * P], src)
                            nc.vector.tensor_copy(osb[:, 3 * P:4 * P], src)
                            nc.gpsimd.dma_start(out.ap()[b, isl, nb * FB:(nb + 1) * FB], osb[:])
                    continue

                # ---- mm1: E[ic] += qT_nn[:,ic,:].T @ kT_nn  over 32 n-chunks ----
                E = [psE_pool.tile([P, FB], f32, tag="E", name=f"E{b}_{i}") for i in range(CP)]
                for nn in range(NN):
                    s, j = nn // SC, nn % SC
                    for ic in range(CP):
                        nc.tensor.matmul(
                            E[ic][:],
                            QT[s][:, j, ic, :],
                            KT[s][:, j, :, :],
                            start=(nn == 0),
                            stop=(nn == NN - 1),
                        )

                # ---- softmax (inverted): att = gamma * exp(m - E) / Z ----
                att = []
                for ic in range(CP):
                    m = small_pool.tile([P, 1], f32, tag="m")
                    nc.vector.tensor_reduce(
                        m[:], E[ic][:], axis=mybir.AxisListType.X,
                        op=mybir.AluOpType.min,
                    )
                    a = att_pool.tile([P, FB], f16, tag="att")
                    z = small_pool.tile([P, 1], f32, tag="z")
                    nc.scalar.activation(
                        a[:], E[ic][:], mybir.ActivationFunctionType.Exp,
                        bias=m[:], scale=-1.0, accum_out=z[:],
                    )
                    zinv = small_pool.tile([P, 1], f32, tag="zi")
                    nc.vector.reciprocal(zinv[:], z[:])
                    asc = small_pool.tile([P, 1], f32, tag="as")
                    nc.vector.tensor_mul(asc[:], zinv[:], gsb[:])
                    nc.vector.tensor_scalar_mul(a[:], a[:], asc[:])
                    att.append(a)

                # ---- transpose att -> attT[j, i] (PE, tiny) ----
                attT = []
                for jc in range(CP):
                    atp = psT_pool.tile([P, FB], f16, tag="wp")
                    jsl = slice(jc * P, (jc + 1) * P)
                    for ic in range(CP):
                        nc.tensor.transpose(
                            atp[:, ic * P:(ic + 1) * P], att[ic][:, jsl], identh[:]
                        )
                    aT = attT_pool.tile([P, FB], f16, tag="attT")
                    if jc % 2 == 0:
                        nc.vector.tensor_copy(aT[:], atp[:])
                    else:
                        nc.scalar.copy(aT[:], atp[:])
                    attT.append(aT)

                # ---- mm2 + residual + store (n-blocks outermost) ----
                for nb in range(NB):
                    for ic in range(CP):
                        isl = slice(ic * P, (ic + 1) * P)
                        acc = psA_pool.tile([P, FB], f32, tag="acc", name=f"acc{b}_{nb}_{ic}")
                        for jc in range(CP):
                            nc.tensor.matmul(
                                acc[:],
                                attT[jc][:, isl],
                                kn_blk(jc, nb * FB, FB),
                                start=(jc == 0),
                                stop=(jc == CP - 1),
                            )
                        osb = out_pool.tile([P, FB], f32, tag="osb")
                        nc.vector.tensor_add(osb[:], acc[:], qn_blk(ic, nb * FB, FB))
                        nc.gpsimd.dma_start(out.ap()[b, isl, nb * FB:(nb + 1) * FB], osb[:])

    nc.compile()
    return nc


def _get_module():
    if "nc" not in _CACHE:
        _CACHE["nc"] = _build()
    return _CACHE["nc"]


def prepare_in_maps(x_high, x_low, gamma):
    x_high = np.asarray(x_high)
    x_low = np.asarray(x_low)
    gamma = np.asarray(gamma, dtype=np.float32).reshape(-1)

    xh3 = x_high.reshape(B, C, N).astype(np.float16)
    xl3 = x_low.reshape(B, C, N).astype(np.float16)
    gm = np.full((P, 1), gamma[0], dtype=np.float32)

    in_maps = []
    for i in range(N_CORES):
        sl = slice(i * B_LOC, (i + 1) * B_LOC)
        in_maps.append({
            "xh": np.ascontiguousarray(xh3[sl]),
            "xl": np.ascontiguousarray(xl3[sl]),
            "gm": gm,
        })
    return in_maps


def kernel(x_high, x_low, gamma):
    from concourse.bass_utils import run_bass_kernel_spmd

    nc = _get_module()
    in_maps = prepare_in_maps(x_high, x_low, gamma)
    res = run_bass_kernel_spmd(nc, in_maps, list(range(N_CORES)))
    out = np.concatenate([res.results[i]["out"] for i in range(N_CORES)], axis=0)
    return out.reshape(B, C, H, W)


# revision 7
# speedup vs baseline: 2.1831x; 2.1831x over previous
"""Trainium2 Bass kernel for CAM (channel attention module).

Reference computation (per batch b):
    q = x_low[b]  as [C, N]   (C=512, N=64*64=4096)
    k = x_high[b] as [C, N]
    E = q @ k.T                              # [C, C]
    att = softmax(rowmax(E) - E, axis=-1)    # == exp(rowmin(E) - E) / Z
    out = gamma * (att @ k) + x_low[b]

Sharding: data-parallel over batch. 16 batches / 8 cores = 2 per core.
gamma is replicated (pre-broadcast on host to [128,1]).

Inputs are cast to fp16 on the host (numerically validated: max rel err
~8.7e-3 vs the fp32 reference, under the 2e-2 gate), halving HBM load
traffic. The n-major layouts needed by the first matmul (contraction
over N) are produced with fp16 PE transposes (1 cycle/row) whose PSUM
results are copied to SBUF on DVE (q) / ACT (k). Matmuls run fp16 in,
fp32 PSUM accumulation. The residual add rides the DVE PSUM->SBUF
eviction of mm2.

tmode="xbar" keeps the DMA x-bar transpose path for comparison (slower:
the x-bar emits 256 B descriptors, ~7 GB/s per DMA engine).
"""

import sys

sys.path.insert(0, "/opt/trn_rl_repo")

import numpy as np

B, C, H, W = 16, 512, 64, 64
N = H * W               # 4096
N_CORES = 8
B_LOC = B // N_CORES    # 2 batches per core
P = 128                 # partitions
CP = C // P             # 4 channel chunks
ST = 1024               # load sub-tile free size (fp16 words)
NS = N // ST            # 4 sub-tiles per (tensor, cc)
SC = ST // P            # 8 n-chunks per sub-tile
NN = N // P             # 32 n chunks of 128
FB = 512                # free-dim block (psum bank) for mm2 output
NB = N // FB            # 8 n blocks of 512

_CACHE = {}


def _build(reps=0, variant="full", tmode="pe"):
    import contextlib
    import concourse.bacc as bacc
    import concourse.tile as tile
    import concourse.mybir as mybir
    from concourse.masks import make_identity

    f32 = mybir.dt.float32
    f16 = mybir.dt.float16

    nc = bacc.Bacc("TRN2", target_bir_lowering=False, debug=False)

    xh = nc.dram_tensor("xh", [B_LOC, C, N], f16, kind="ExternalInput")
    xl = nc.dram_tensor("xl", [B_LOC, C, N], f16, kind="ExternalInput")
    gm = nc.dram_tensor("gm", [P, 1], f32, kind="ExternalInput")
    out = nc.dram_tensor("out", [B_LOC, C, N], f32, kind="ExternalOutput")

    if tmode == "pe":
        kTb, qTb = 6, 6          # per-nn [P, CP*P] fp16 tiles
        knb = qnb = 2 * NS * CP  # full double-buffer of naturals
        psTb = 2                 # PE-transpose staging (q/k, bank-granular)
    else:
        kTb = qTb = NS - 1       # [P, SC, CP, P] s-group tiles
        knb = qnb = 2 * NS * CP - 4
        psTb = 2

    with tile.TileContext(nc) as tc:
        with (
            tc.tile_pool(name="const", bufs=1) as const_pool,
            tc.tile_pool(name="kn", bufs=knb) as kn_pool,
            tc.tile_pool(name="qn", bufs=qnb) as qn_pool,
            tc.tile_pool(name="kT", bufs=kTb) as kT_pool,
            tc.tile_pool(name="qT", bufs=qTb) as qT_pool,
            tc.tile_pool(name="att", bufs=CP) as att_pool,
            tc.tile_pool(name="attT", bufs=CP) as attT_pool,
            tc.tile_pool(name="osb", bufs=2) as out_pool,
            tc.tile_pool(name="small", bufs=24) as small_pool,
            tc.tile_pool(name="psE", bufs=CP, space="PSUM") as psE_pool,
            tc.tile_pool(name="psT", bufs=psTb, space="PSUM") as psT_pool,
            tc.tile_pool(name="psA", bufs=2, space="PSUM") as psA_pool,
        ):
            ident_f = const_pool.tile([P, P], f32)
            make_identity(nc, ident_f[:])
            identh = const_pool.tile([P, P], f16)
            nc.vector.tensor_copy(identh[:], ident_f[:])
            gsb = const_pool.tile([P, 1], f32)
            nc.sync.dma_start(gsb[:], gm.ap())

            rep_ctx = tc.For_i(0, reps, 1) if reps else contextlib.nullcontext()
            with rep_ctx:
              for b in range(B_LOC):
                KN = [[None] * NS for _ in range(CP)]
                QN = [[None] * NS for _ in range(CP)]
                KT = [None] * max(NS, NN)
                QT = [None] * max(NS, NN)

                def emit_xbar(s):
                    # [128c, ST n] -> [128 n, SC, 128 c] per channel chunk:
                    # T[s][p, j, cc, c] = nat[cc*128+c, (s*SC+j)*128 + p]
                    kTs = kT_pool.tile([P, SC, CP, P], f16, tag="kT", name=f"kT{b}_{s}")
                    qTs = qT_pool.tile([P, SC, CP, P], f16, tag="qT", name=f"qT{b}_{s}")
                    for cc in range(CP):
                        nc.sync.dma_start(kTs[:, :, cc, :], KN[cc][s][:], transpose=True)
                        nc.sync.dma_start(qTs[:, :, cc, :], QN[cc][s][:], transpose=True)
                    KT[s] = kTs
                    QT[s] = qTs

                for s in range(NS):
                    ssl = slice(s * ST, (s + 1) * ST)
                    for cc in range(CP):
                        csl = slice(cc * P, (cc + 1) * P)
                        kt = kn_pool.tile([P, ST], f16, tag="kn", name=f"kn{b}_{cc}_{s}")
                        qt = qn_pool.tile([P, ST], f16, tag="qn", name=f"qn{b}_{cc}_{s}")
                        nc.sync.dma_start(kt[:], xh.ap()[b, csl, ssl])
                        nc.sync.dma_start(qt[:], xl.ap()[b, csl, ssl])
                        KN[cc][s] = kt
                        QN[cc][s] = qt
                    if tmode == "xbar" and variant != "dma" and s > 0:
                        emit_xbar(s - 1)
                if tmode == "xbar" and variant != "dma":
                    emit_xbar(NS - 1)

                def qn_blk(cc, lo, width):
                    s = lo // ST
                    o = lo - s * ST
                    return QN[cc][s][:, o:o + width]

                def kn_blk(cc, lo, width):
                    s = lo // ST
                    o = lo - s * ST
                    return KN[cc][s][:, o:o + width]

                if variant == "dma":
                    # loads + stores only: measures the pure HBM floor
                    for nb in range(NB):
                        osb = out_pool.tile([P, CP, FB], f32, tag="osb")
                        for ic in range(CP):
                            src = qn_blk(ic, nb * FB, FB) if nb % 2 == 0 else kn_blk(ic, nb * FB, FB)
                            nc.vector.tensor_copy(osb[:, ic, :], src)
                        dst = out.ap()[b].rearrange("(i p) n -> p i n", p=P)
                        nc.gpsimd.dma_start(dst[:, :, nb * FB:(nb + 1) * FB], osb[:])
                    continue

                # ---- transposes + mm1: E[ic] += qT_nn[:,ic,:].T @ kT_nn ----
                E = [psE_pool.tile([P, FB], f32, tag="E", name=f"E{b}_{i}") for i in range(CP)]

                if tmode == "pe":
                    def emit_T(nn):
                        # PE-transpose all 4 cc blocks of q and k for n-chunk
                        # nn into PSUM, then evict to SBUF (DVE=q, ACT=k).
                        qtp = psT_pool.tile([P, FB], f16, tag="wp")
                        ktp = psT_pool.tile([P, FB], f16, tag="wp")
                        for cc in range(CP):
                            csl = slice(cc * P, (cc + 1) * P)
                            nc.tensor.transpose(
                                qtp[:, csl], qn_blk(cc, nn * P, P), identh[:])
                            nc.tensor.transpose(
                                ktp[:, csl], kn_blk(cc, nn * P, P), identh[:])
                        qTn = qT_pool.tile([P, FB], f16, tag="qT", name=f"qT{b}_{nn}")
                        nc.vector.tensor_copy(qTn[:], qtp[:])
                        kTn = kT_pool.tile([P, FB], f16, tag="kT", name=f"kT{b}_{nn}")
                        nc.scalar.copy(kTn[:], ktp[:])
                        QT[nn] = qTn
                        KT[nn] = kTn

                    def emit_mm1(nn):
                        for ic in range(CP):
                            nc.tensor.matmul(
                                E[ic][:],
                                QT[nn][:, ic * P:(ic + 1) * P],
                                KT[nn][:],
                                start=(nn == 0),
                                stop=(nn == NN - 1),
                            )

                    emit_T(0)
                    for nn in range(1, NN):
                        emit_T(nn)
                        emit_mm1(nn - 1)
                    emit_mm1(NN - 1)
                else:
                    for nn in range(NN):
                        s, j = nn // SC, nn % SC
                        for ic in range(CP):
                            nc.tensor.matmul(
                                E[ic][:],
                                QT[s][:, j, ic, :],
                                KT[s][:, j, :, :],
                                start=(nn == 0),
                                stop=(nn == NN - 1),
                            )

                # ---- softmax (inverted): att = gamma * exp(m - E) / Z ----
                att = []
                for ic in range(CP):
                    m = small_pool.tile([P, 1], f32, tag="m")
                    nc.vector.tensor_reduce(
                        m[:], E[ic][:], axis=mybir.AxisListType.X,
                        op=mybir.AluOpType.min,
                    )
                    a = att_pool.tile([P, FB], f16, tag="att")
                    z = small_pool.tile([P, 1], f32, tag="z")
                    nc.scalar.activation(
                        a[:], E[ic][:], mybir.ActivationFunctionType.Exp,
                        bias=m[:], scale=-1.0, accum_out=z[:],
                    )
                    zinv = small_pool.tile([P, 1], f32, tag="zi")
                    nc.vector.reciprocal(zinv[:], z[:])
                    asc = small_pool.tile([P, 1], f32, tag="as")
                    nc.vector.tensor_mul(asc[:], zinv[:], gsb[:])
                    nc.vector.tensor_scalar_mul(a[:], a[:], asc[:])
                    att.append(a)

                # ---- transpose att -> attT[j, i] (PE, tiny) ----
                attT = []
                for jc in range(CP):
                    atp = psT_pool.tile([P, FB], f16, tag="wp")
                    jsl = slice(jc * P, (jc + 1) * P)
                    for ic in range(CP):
                        nc.tensor.transpose(
                            atp[:, ic * P:(ic + 1) * P], att[ic][:, jsl], identh[:]
                        )
                    aT = attT_pool.tile([P, FB], f16, tag="attT")
                    if jc % 2 == 0:
                        nc.vector.tensor_copy(aT[:], atp[:])
                    else:
                        nc.scalar.copy(aT[:], atp[:])
                    attT.append(aT)

                # ---- mm2 + residual + store (n-blocks outermost) ----
                for nb in range(NB):
                    osb = out_pool.tile([P, CP, FB], f32, tag="osb")
                    for ic in range(CP):
                        isl = slice(ic * P, (ic + 1) * P)
                        acc = psA_pool.tile([P, FB], f32, tag="acc", name=f"acc{b}_{nb}_{ic}")
                        for jc in range(CP):
                            nc.tensor.matmul(
                                acc[:],
                                attT[jc][:, isl],
                                kn_blk(jc, nb * FB, FB),
                                start=(jc == 0),
                                stop=(jc == CP - 1),
                            )
                        nc.vector.tensor_add(osb[:, ic, :], acc[:], qn_blk(ic, nb * FB, FB))
                    dst = out.ap()[b].rearrange("(i p) n -> p i n", p=P)
                    nc.gpsimd.dma_start(dst[:, :, nb * FB:(nb + 1) * FB], osb[:])

    nc.compile()
    return nc


def _get_module():
    if "nc" not in _CACHE:
        _CACHE["nc"] = _build()
    return _CACHE["nc"]


def prepare_in_maps(x_high, x_low, gamma):
    x_high = np.asarray(x_high)
    x_low = np.asarray(x_low)
    gamma = np.asarray(gamma, dtype=np.float32).reshape(-1)

    xh3 = x_high.reshape(B, C, N).astype(np.float16)
    xl3 = x_low.reshape(B, C, N).astype(np.float16)
    gm = np.full((P, 1), gamma[0], dtype=np.float32)

    in_maps = []
    for i in range(N_CORES):
        sl = slice(i * B_LOC, (i + 1) * B_LOC)
        in_maps.append({
            "xh": np.ascontiguousarray(xh3[sl]),
            "xl": np.ascontiguousarray(xl3[sl]),
            "gm": gm,
        })
    return in_maps


def kernel(x_high, x_low, gamma):
    from concourse.bass_utils import run_bass_kernel_spmd

    nc = _get_module()
    in_maps = prepare_in_maps(x_high, x_low, gamma)
    res = run_bass_kernel_spmd(nc, in_maps, list(range(N_CORES)))
    out = np.concatenate([res.results[i]["out"] for i in range(N_CORES)], axis=0)
    return out.reshape(B, C, H, W)


# revision 9
# speedup vs baseline: 2.3229x; 1.0640x over previous
"""Trainium2 Bass kernel for CAM (channel attention module).

Reference computation (per batch b):
    q = x_low[b]  as [C, N]   (C=512, N=64*64=4096)
    k = x_high[b] as [C, N]
    E = q @ k.T                              # [C, C]
    att = softmax(rowmax(E) - E, axis=-1)    # == exp(rowmin(E) - E) / Z
    out = gamma * (att @ k) + x_low[b]

Sharding: data-parallel over batch. 16 batches / 8 cores = 2 per core.
gamma is replicated (pre-broadcast on host to [128,1]).

Inputs are cast to fp16 on the host (numerically validated: max rel err
~8.7e-3 vs the fp32 reference, under the 2e-2 gate), halving HBM load
traffic. The n-major layouts needed by the first matmul (contraction
over N) are produced with fp16 PE transposes (1 cycle/row) whose PSUM
results are copied to SBUF on DVE (q) / ACT (k). Matmuls run fp16 in,
fp32 PSUM accumulation. The residual add rides the DVE PSUM->SBUF
eviction of mm2.

The two batches are software-pipelined on the PE queue: batch b+1's
transposes+mm1 are interleaved with batch b's attT/mm2 so the PE never
waits on softmax (DVE/ACT) or output drain (DVE/DMA).
"""

import sys

sys.path.insert(0, "/opt/trn_rl_repo")

import numpy as np

B, C, H, W = 16, 512, 64, 64
N = H * W               # 4096
N_CORES = 8
B_LOC = B // N_CORES    # 2 batches per core
P = 128                 # partitions
CP = C // P             # 4 channel chunks
ST = 1024               # load sub-tile free size (fp16 words)
NS = N // ST            # 4 sub-tiles per (tensor, cc)
NN = N // P             # 32 n chunks of 128
FB = 512                # free-dim block (psum bank) for mm2 output
NB = N // FB            # 8 n blocks of 512

_CACHE = {}


def _build(reps=0, variant="full"):
    import contextlib
    import concourse.bacc as bacc
    import concourse.tile as tile
    import concourse.mybir as mybir
    from concourse.masks import make_identity

    f32 = mybir.dt.float32
    f16 = mybir.dt.float16

    nc = bacc.Bacc("TRN2", target_bir_lowering=False, debug=False)

    xh = nc.dram_tensor("xh", [B_LOC, C, N], f16, kind="ExternalInput")
    xl = nc.dram_tensor("xl", [B_LOC, C, N], f16, kind="ExternalInput")
    gm = nc.dram_tensor("gm", [P, 1], f32, kind="ExternalInput")
    out = nc.dram_tensor("out", [B_LOC, C, N], f32, kind="ExternalOutput")

    with tile.TileContext(nc) as tc:
        with (
            tc.tile_pool(name="const", bufs=1) as const_pool,
            tc.tile_pool(name="kn", bufs=2 * NS * CP) as kn_pool,
            tc.tile_pool(name="qn", bufs=2 * NS * CP) as qn_pool,
            tc.tile_pool(name="kT", bufs=6) as kT_pool,
            tc.tile_pool(name="qT", bufs=6) as qT_pool,
            tc.tile_pool(name="att", bufs=2 * CP) as att_pool,
            tc.tile_pool(name="attT", bufs=CP) as attT_pool,
            tc.tile_pool(name="osb", bufs=3) as out_pool,
            tc.tile_pool(name="small", bufs=32) as small_pool,
            tc.tile_pool(name="psE", bufs=CP, space="PSUM") as psE_pool,
            tc.tile_pool(name="psT", bufs=2, space="PSUM") as psT_pool,
            tc.tile_pool(name="psA", bufs=2, space="PSUM") as psA_pool,
        ):
            ident_f = const_pool.tile([P, P], f32)
            make_identity(nc, ident_f[:])
            identh = const_pool.tile([P, P], f16)
            nc.vector.tensor_copy(identh[:], ident_f[:])
            gsb = const_pool.tile([P, 1], f32)
            nc.sync.dma_start(gsb[:], gm.ap())

            class BatchState:
                pass

            def make_state(b):
                st = BatchState()
                st.b = b
                st.KN = [[None] * NS for _ in range(CP)]
                st.QN = [[None] * NS for _ in range(CP)]
                st.KT = [None] * NN
                st.QT = [None] * NN
                st.E = None
                st.att = []
                st.attT = []
                return st

            def emit_loads(st):
                b = st.b
                for s in range(NS):
                    ssl = slice(s * ST, (s + 1) * ST)
                    for cc in range(CP):
                        csl = slice(cc * P, (cc + 1) * P)
                        kt = kn_pool.tile([P, ST], f16, tag="kn", name=f"kn{b}_{cc}_{s}")
                        qt = qn_pool.tile([P, ST], f16, tag="qn", name=f"qn{b}_{cc}_{s}")
                        nc.sync.dma_start(kt[:], xh.ap()[b, csl, ssl])
                        nc.sync.dma_start(qt[:], xl.ap()[b, csl, ssl])
                        st.KN[cc][s] = kt
                        st.QN[cc][s] = qt

            def nat_blk(tiles, cc, lo, width):
                s = lo // ST
                o = lo - s * ST
                return tiles[cc][s][:, o:o + width]

            def gen_tmm1(st):
                """33 steps: PE transposes of n-chunk nn + mm1 of chunk nn-1."""
                b = st.b
                st.E = [psE_pool.tile([P, FB], f32, tag="E", name=f"E{b}_{i}")
                        for i in range(CP)]

                def emit_T(nn):
                    qtp = psT_pool.tile([P, FB], f16, tag="wp")
                    ktp = psT_pool.tile([P, FB], f16, tag="wp")
                    for cc in range(CP):
                        csl = slice(cc * P, (cc + 1) * P)
                        nc.tensor.transpose(
                            qtp[:, csl], nat_blk(st.QN, cc, nn * P, P), identh[:])
                        nc.tensor.transpose(
                            ktp[:, csl], nat_blk(st.KN, cc, nn * P, P), identh[:])
                    qTn = qT_pool.tile([P, FB], f16, tag="qT", name=f"qT{b}_{nn}")
                    nc.vector.tensor_copy(qTn[:], qtp[:])
                    kTn = kT_pool.tile([P, FB], f16, tag="kT", name=f"kT{b}_{nn}")
                    nc.scalar.copy(kTn[:], ktp[:])
                    st.QT[nn] = qTn
                    st.KT[nn] = kTn

                def emit_mm1(nn):
                    for ic in range(CP):
                        nc.tensor.matmul(
                            st.E[ic][:],
                            st.QT[nn][:, ic * P:(ic + 1) * P],
                            st.KT[nn][:],
                            start=(nn == 0),
                            stop=(nn == NN - 1),
                        )

                for nn in range(NN):
                    emit_T(nn)
                    if nn > 0:
                        emit_mm1(nn - 1)
                    yield
                emit_mm1(NN - 1)
                yield

            def emit_softmax(st):
                # DVE/ACT only -- no PE instructions
                for ic in range(CP):
                    m = small_pool.tile([P, 1], f32, tag="m")
                    nc.vector.tensor_reduce(
                        m[:], st.E[ic][:], axis=mybir.AxisListType.X,
                        op=mybir.AluOpType.min,
                    )
                    a = att_pool.tile([P, FB], f16, tag="att")
                    z = small_pool.tile([P, 1], f32, tag="z")
                    nc.scalar.activation(
                        a[:], st.E[ic][:], mybir.ActivationFunctionType.Exp,
                        bias=m[:], scale=-1.0, accum_out=z[:],
                    )
                    zinv = small_pool.tile([P, 1], f32, tag="zi")
                    nc.vector.reciprocal(zinv[:], z[:])
                    asc = small_pool.tile([P, 1], f32, tag="as")
                    nc.vector.tensor_mul(asc[:], zinv[:], gsb[:])
                    nc.vector.tensor_scalar_mul(a[:], a[:], asc[:])
                    st.att.append(a)

            def emit_attT(st):
                for jc in range(CP):
                    atp = psT_pool.tile([P, FB], f16, tag="wp")
                    jsl = slice(jc * P, (jc + 1) * P)
                    for ic in range(CP):
                        nc.tensor.transpose(
                            atp[:, ic * P:(ic + 1) * P], st.att[ic][:, jsl], identh[:]
                        )
                    aT = attT_pool.tile([P, FB], f16, tag="attT")
                    if jc % 2 == 0:
                        nc.vector.tensor_copy(aT[:], atp[:])
                    else:
                        nc.scalar.copy(aT[:], atp[:])
                    st.attT.append(aT)

            def gen_mm2(st):
                """32 steps: one (nb, ic) accumulation + residual; store per nb."""
                b = st.b
                dst = out.ap()[b].rearrange("(i p) n -> p i n", p=P)
                for nb in range(NB):
                    osb = out_pool.tile([P, CP, FB], f32, tag="osb")
                    for ic in range(CP):
                        isl = slice(ic * P, (ic + 1) * P)
                        acc = psA_pool.tile([P, FB], f32, tag="acc",
                                            name=f"acc{b}_{nb}_{ic}")
                        for jc in range(CP):
                            nc.tensor.matmul(
                                acc[:],
                                st.attT[jc][:, isl],
                                nat_blk(st.KN, jc, nb * FB, FB),
                                start=(jc == 0),
                                stop=(jc == CP - 1),
                            )
                        nc.vector.tensor_add(
                            osb[:, ic, :], acc[:], nat_blk(st.QN, ic, nb * FB, FB))
                        yield
                    nc.gpsimd.dma_start(dst[:, :, nb * FB:(nb + 1) * FB], osb[:])

            def emit_dma_variant(st):
                b = st.b
                dst = out.ap()[b].rearrange("(i p) n -> p i n", p=P)
                for nb in range(NB):
                    osb = out_pool.tile([P, CP, FB], f32, tag="osb")
                    for ic in range(CP):
                        src = (nat_blk(st.QN, ic, nb * FB, FB) if nb % 2 == 0
                               else nat_blk(st.KN, ic, nb * FB, FB))
                        nc.vector.tensor_copy(osb[:, ic, :], src)
                    nc.gpsimd.dma_start(dst[:, :, nb * FB:(nb + 1) * FB], osb[:])

            def drain(g, n=None):
                i = 0
                for _ in g:
                    i += 1
                    if n is not None and i >= n:
                        return True
                return False

            rep_ctx = tc.For_i(0, reps, 1) if reps else contextlib.nullcontext()
            with rep_ctx:
                states = [make_state(b) for b in range(B_LOC)]
                for st in states:
                    emit_loads(st)

                if variant == "dma":
                    for st in states:
                        emit_dma_variant(st)
                else:
                    # software-pipelined schedule over the two batches
                    s0, s1 = states
                    g0 = gen_tmm1(s0)
                    drain(g0)                 # batch0 transposes + mm1
                    emit_softmax(s0)          # DVE/ACT; PE continues below
                    g1 = gen_tmm1(s1)
                    drain(g1, 3)              # fill batch0's softmax latency
                    emit_attT(s0)
                    m0 = gen_mm2(s0)
                    t = 3
                    more = True
                    while more:
                        more = drain(g1, 1)
                        t += 1
                        if t % 4 != 0:
                            drain(m0, 1)
                    emit_softmax(s1)          # DVE/ACT run during m0 leftover
                    drain(m0)                 # PE: leftover mm2 fills softmax gap
                    emit_attT(s1)
                    drain(gen_mm2(s1))

    nc.compile()
    return nc


def _get_module():
    if "nc" not in _CACHE:
        _CACHE["nc"] = _build()
    return _CACHE["nc"]


def prepare_in_maps(x_high, x_low, gamma):
    x_high = np.asarray(x_high)
    x_low = np.asarray(x_low)
    gamma = np.asarray(gamma, dtype=np.float32).reshape(-1)

    xh3 = x_high.reshape(B, C, N).astype(np.float16)
    xl3 = x_low.reshape(B, C, N).astype(np.float16)
    gm = np.full((P, 1), gamma[0], dtype=np.float32)

    in_maps = []
    for i in range(N_CORES):
        sl = slice(i * B_LOC, (i + 1) * B_LOC)
        in_maps.append({
            "xh": np.ascontiguousarray(xh3[sl]),
            "xl": np.ascontiguousarray(xl3[sl]),
            "gm": gm,
        })
    return in_maps


def kernel(x_high, x_low, gamma):
    from concourse.bass_utils import run_bass_kernel_spmd

    nc = _get_module()
    in_maps = prepare_in_maps(x_high, x_low, gamma)
    res = run_bass_kernel_spmd(nc, in_maps, list(range(N_CORES)))
    out = np.concatenate([res.results[i]["out"] for i in range(N_CORES)], axis=0)
    return out.reshape(B, C, H, W)


# revision 14
# speedup vs baseline: 3.7501x; 1.6144x over previous
"""Trainium2 Bass kernel for CAM (channel attention module).

Reference computation (per batch b):
    q = x_low[b]  as [C, N]   (C=512, N=64*64=4096)
    k = x_high[b] as [C, N]
    E = q @ k.T                              # [C, C]
    att = softmax(rowmax(E) - E, axis=-1)    # == exp(rowmin(E) - E) / Z
    out = gamma * (att @ k) + x_low[b]

Sharding: data-parallel over batch. 16 batches / 8 cores = 2 per core.
gamma is replicated (pre-broadcast on host to [128,1]).

Inputs are cast to fp16 on the host (numerically validated: max rel err
~8.7e-3 vs the fp32 reference, under the 2e-2 gate), halving HBM load
traffic. The n-major layouts needed by the first matmul (contraction
over N) are produced with fp16 PE transposes (1 cycle/row) whose PSUM
results are copied to SBUF on DVE (q) / ACT (k). Matmuls run fp16 in,
fp32 PSUM accumulation. The residual add rides the DVE PSUM->SBUF
eviction of mm2.

The two batches are software-pipelined on the PE queue: batch b+1's
transposes+mm1 are interleaved with batch b's attT/mm2 so the PE never
waits on softmax (DVE/ACT) or output drain (DVE/DMA).
"""

import sys

sys.path.insert(0, "/opt/trn_rl_repo")

import numpy as np

B, C, H, W = 16, 512, 64, 64
N = H * W               # 4096
N_CORES = 8
B_LOC = B // N_CORES    # 2 batches per core
P = 128                 # partitions
CP = C // P             # 4 channel chunks
ST = 1024               # load sub-tile free size (fp16 words)
NS = N // ST            # 4 sub-tiles per (tensor, cc)
NN = N // P             # 32 n chunks of 128
FB = 512                # free-dim block (psum bank) for mm2 output
NB = N // FB            # 8 n blocks of 512

_CACHE = {}


def _build(reps=0, variant="full"):
    import contextlib
    import concourse.bacc as bacc
    import concourse.tile as tile
    import concourse.mybir as mybir
    from concourse.masks import make_identity

    f32 = mybir.dt.float32
    f16 = mybir.dt.float16

    nc = bacc.Bacc("TRN2", target_bir_lowering=False, debug=False)

    xh = nc.dram_tensor("xh", [B_LOC, C, N], f16, kind="ExternalInput")
    xl = nc.dram_tensor("xl", [B_LOC, C, N], f16, kind="ExternalInput")
    gm = nc.dram_tensor("gm", [P, 1], f32, kind="ExternalInput")
    out = nc.dram_tensor("out", [B_LOC, C, N], f32, kind="ExternalOutput")

    with tile.TileContext(nc) as tc:
        with (
            tc.tile_pool(name="const", bufs=1) as const_pool,
            tc.tile_pool(name="kn", bufs=2 * NS * CP) as kn_pool,
            tc.tile_pool(name="qn", bufs=2 * NS * CP) as qn_pool,
            tc.tile_pool(name="kT", bufs=6) as kT_pool,
            tc.tile_pool(name="qT", bufs=6) as qT_pool,
            tc.tile_pool(name="att", bufs=2 * CP) as att_pool,
            tc.tile_pool(name="attT", bufs=CP) as attT_pool,
            tc.tile_pool(name="osb", bufs=4) as out_pool,
            tc.tile_pool(name="small", bufs=32) as small_pool,
            tc.tile_pool(name="psE", bufs=CP, space="PSUM") as psE_pool,
            tc.tile_pool(name="psT", bufs=2, space="PSUM") as psT_pool,
            tc.tile_pool(name="psA", bufs=2, space="PSUM") as psA_pool,
        ):
            ident_f = const_pool.tile([P, P], f32)
            make_identity(nc, ident_f[:])
            identh = const_pool.tile([P, P], f16)
            nc.vector.tensor_copy(identh[:], ident_f[:])
            gsb = const_pool.tile([P, 1], f32)
            nc.sync.dma_start(gsb[:], gm.ap())

            class BatchState:
                pass

            def make_state(b):
                st = BatchState()
                st.b = b
                st.KN = [[None] * NS for _ in range(CP)]
                st.QN = [[None] * NS for _ in range(CP)]
                st.KT = [None] * NN
                st.QT = [None] * NN
                st.E = None
                st.att = []
                st.attT = []
                return st

            def emit_loads(st, split_first=False):
                b = st.b
                for s in range(NS):
                    ssl = slice(s * ST, (s + 1) * ST)
                    for cc in range(CP):
                        csl = slice(cc * P, (cc + 1) * P)
                        kt = kn_pool.tile([P, ST], f16, tag="kn", name=f"kn{b}_{cc}_{s}")
                        qt = qn_pool.tile([P, ST], f16, tag="qn", name=f"qn{b}_{cc}_{s}")
                        nc.sync.dma_start(kt[:], xh.ap()[b, csl, ssl])
                        nc.sync.dma_start(qt[:], xl.ap()[b, csl, ssl])
                        st.KN[cc][s] = kt
                        st.QN[cc][s] = qt

            def nat_blk(tiles, cc, lo, width):
                s = lo // ST
                o = lo - s * ST
                return tiles[cc][s][:, o:o + width]

            def gen_tmm1(st):
                """33 steps: PE transposes of n-chunk nn + mm1 of chunk nn-1."""
                b = st.b
                st.E = [psE_pool.tile([P, FB], f32, tag="E", name=f"E{b}_{i}")
                        for i in range(CP)]

                def emit_T(nn):
                    qtp = psT_pool.tile([P, FB], f16, tag="wp")
                    ktp = psT_pool.tile([P, FB], f16, tag="wp")
                    for cc in range(CP):
                        csl = slice(cc * P, (cc + 1) * P)
                        nc.tensor.transpose(
                            qtp[:, csl], nat_blk(st.QN, cc, nn * P, P), identh[:])
                        nc.tensor.transpose(
                            ktp[:, csl], nat_blk(st.KN, cc, nn * P, P), identh[:])
                    qTn = qT_pool.tile([P, FB], f16, tag="qT", name=f"qT{b}_{nn}")
                    nc.vector.tensor_copy(qTn[:], qtp[:])
                    kTn = kT_pool.tile([P, FB], f16, tag="kT", name=f"kT{b}_{nn}")
                    nc.scalar.copy(kTn[:], ktp[:])
                    st.QT[nn] = qTn
                    st.KT[nn] = kTn

                def emit_mm1(nn):
                    for ic in range(CP):
                        nc.tensor.matmul(
                            st.E[ic][:],
                            st.QT[nn][:, ic * P:(ic + 1) * P],
                            st.KT[nn][:],
                            start=(nn == 0),
                            stop=(nn == NN - 1),
                        )

                for nn in range(NN):
                    emit_T(nn)
                    if nn > 0:
                        emit_mm1(nn - 1)
                    yield
                emit_mm1(NN - 1)
                yield

            def emit_softmax(st):
                # DVE/ACT only -- no PE instructions
                for ic in range(CP):
                    m = small_pool.tile([P, 1], f32, tag="m")
                    nc.vector.tensor_reduce(
                        m[:], st.E[ic][:], axis=mybir.AxisListType.X,
                        op=mybir.AluOpType.min,
                    )
                    a = att_pool.tile([P, FB], f16, tag="att")
                    z = small_pool.tile([P, 1], f32, tag="z")
                    nc.scalar.activation(
                        a[:], st.E[ic][:], mybir.ActivationFunctionType.Exp,
                        bias=m[:], scale=-1.0, accum_out=z[:],
                    )
                    zinv = small_pool.tile([P, 1], f32, tag="zi")
                    nc.vector.reciprocal(zinv[:], z[:])
                    asc = small_pool.tile([P, 1], f32, tag="as")
                    nc.vector.tensor_mul(asc[:], zinv[:], gsb[:])
                    nc.vector.tensor_scalar_mul(a[:], a[:], asc[:])
                    st.att.append(a)

            def emit_attT(st):
                for jc in range(CP):
                    atp = psT_pool.tile([P, FB], f16, tag="wp")
                    jsl = slice(jc * P, (jc + 1) * P)
                    for ic in range(CP):
                        nc.tensor.transpose(
                            atp[:, ic * P:(ic + 1) * P], st.att[ic][:, jsl], identh[:]
                        )
                    aT = attT_pool.tile([P, FB], f16, tag="attT")
                    if jc % 2 == 0:
                        nc.vector.tensor_copy(aT[:], atp[:])
                    else:
                        nc.scalar.copy(aT[:], atp[:])
                    st.attT.append(aT)

            def gen_mm2(st, split_last=0):
                """32 steps: one (nb, ic) accumulation + residual; store per nb.
                The last `split_last` n-blocks store per-ic so the drain tail
                after the final matmul is a 256 KiB transfer, not 1 MiB."""
                b = st.b
                dst = out.ap()[b].rearrange("(i p) n -> p i n", p=P)
                for nb in range(NB):
                    fine = nb >= NB - split_last
                    osb = out_pool.tile([P, CP, FB], f32, tag="osb")
                    for ic in range(CP):
                        isl = slice(ic * P, (ic + 1) * P)
                        acc = psA_pool.tile([P, FB], f32, tag="acc",
                                            name=f"acc{b}_{nb}_{ic}")
                        for jc in range(CP):
                            nc.tensor.matmul(
                                acc[:],
                                st.attT[jc][:, isl],
                                nat_blk(st.KN, jc, nb * FB, FB),
                                start=(jc == 0),
                                stop=(jc == CP - 1),
                            )
                        nc.vector.tensor_add(
                            osb[:, ic, :], acc[:], nat_blk(st.QN, ic, nb * FB, FB))
                        if fine:
                            nc.gpsimd.dma_start(
                                dst[:, ic:ic + 1, nb * FB:(nb + 1) * FB],
                                osb[:, ic:ic + 1, :])
                        yield
                    if not fine:
                        nc.gpsimd.dma_start(dst[:, :, nb * FB:(nb + 1) * FB], osb[:])

            def emit_dma_variant(st):
                b = st.b
                dst = out.ap()[b].rearrange("(i p) n -> p i n", p=P)
                for nb in range(NB):
                    osb = out_pool.tile([P, CP, FB], f32, tag="osb")
                    for ic in range(CP):
                        src = (nat_blk(st.QN, ic, nb * FB, FB) if nb % 2 == 0
                               else nat_blk(st.KN, ic, nb * FB, FB))
                        nc.vector.tensor_copy(osb[:, ic, :], src)
                    nc.gpsimd.dma_start(dst[:, :, nb * FB:(nb + 1) * FB], osb[:])

            def drain(g, n=None):
                i = 0
                for _ in g:
                    i += 1
                    if n is not None and i >= n:
                        return True
                return False

            rep_ctx = tc.For_i(0, reps, 1) if reps else contextlib.nullcontext()
            with rep_ctx:
                states = [make_state(b) for b in range(B_LOC)]
                for st in states:
                    emit_loads(st)

                if variant == "dma":
                    for st in states:
                        emit_dma_variant(st)
                else:
                    # software-pipelined schedule over the two batches
                    s0, s1 = states
                    g0 = gen_tmm1(s0)
                    drain(g0)                 # batch0 transposes + mm1
                    emit_softmax(s0)          # DVE/ACT; PE continues below
                    g1 = gen_tmm1(s1)
                    drain(g1, 4)              # fill batch0's softmax latency
                    emit_attT(s0)
                    m0 = gen_mm2(s0)
                    t = 4
                    more = True
                    while more:
                        more = drain(g1, 1)
                        t += 1
                        if t % 4 != 0:
                            drain(m0, 1)
                    emit_softmax(s1)          # DVE/ACT run during m0 leftover
                    drain(m0)                 # PE: leftover mm2 fills softmax gap
                    emit_attT(s1)
                    drain(gen_mm2(s1, split_last=2))

    nc.compile()
    return nc


def _get_module():
    if "nc" not in _CACHE:
        _CACHE["nc"] = _build()
    return _CACHE["nc"]


def prepare_in_maps(x_high, x_low, gamma):
    x_high = np.asarray(x_high)
    x_low = np.asarray(x_low)
    gamma = np.asarray(gamma, dtype=np.float32).reshape(-1)

    xh3 = x_high.reshape(B, C, N).astype(np.float16)
    xl3 = x_low.reshape(B, C, N).astype(np.float16)
    gm = np.full((P, 1), gamma[0], dtype=np.float32)

    in_maps = []
    for i in range(N_CORES):
        sl = slice(i * B_LOC, (i + 1) * B_LOC)
        in_maps.append({
            "xh": np.ascontiguousarray(xh3[sl]),
            "xl": np.ascontiguousarray(xl3[sl]),
            "gm": gm,
        })
    return in_maps


def kernel(x_high, x_low, gamma):
    from concourse.bass_utils import run_bass_kernel_spmd

    nc = _get_module()
    in_maps = prepare_in_maps(x_high, x_low, gamma)
    res = run_bass_kernel_spmd(nc, in_maps, list(range(N_CORES)))
    out = np.concatenate([res.results[i]["out"] for i in range(N_CORES)], axis=0)
    return out.reshape(B, C, H, W)
